# revision 46
# baseline (speedup 1.0000x reference)
"""NeuronPool (moe_routing) Trainium2 kernel.

Expert-parallel over 8 NeuronCores: core c computes neurons [8c, 8c+8) for the
full batch, host concatenates along the neuron axis.

The kernel is HBM-bound: ~25 MB of fp16 weights stream per core at ~420 GB/s
(weights cast on host; matmuls run at the full 1 cycle/row PE rate, ~5e-4
relative rounding).  The per-neuron MLPs are software-pipelined one neuron
deep so the PE never waits on gelu: step n runs GEMM1(n) while GEMM2/GEMM3 of
neuron n-1 fill the gelu/transpose latency.

  x = [proj | hist]: proj = Wp.T @ embT on the PE; hist chunks broadcast
      across batch on the DVE.  W1/W2 weights stream on the fast gpsimd SWDGE
      ring (16 DMA engines); small constants ride the sync ring.
  step n:  G2(n-1) ; G1(n) ; tr4 h2(n-1) ; G3(n-1) ; tr4 h1(n), with the
           two gelus retiring on the scalar engine in consumption order
  GEMM3 output is [32,257]: col 256 = row-sums of W3 (host-augmented), so the
  LayerNorm mean is a free by-product of the GEMM.
  LN scale: sqrt over neurons {0..6} runs while neuron 7 computes (their
  outputs drain early); only neuron 7 remains in the tail.  The oscillator
  mod is folded into inv_std via one DVE multiply with a per-core constant.
Zero bias vectors / unit gamma / zero beta are detected on host at call time
and their device ops are skipped (generic selector-matmul paths are emitted
when the values are non-trivial).
"""
import math
import numpy as np
from contextlib import ExitStack

import concourse.bass as bass
import concourse.tile as tile
from concourse import bacc, mybir
from concourse.bass_utils import run_bass_kernel_spmd

N_CORES = 8
B = 32          # batch
D = 256         # model dim
HIST = 8
HID = 512
N_NEURONS = 64
NPC = N_NEURONS // N_CORES  # 8 neurons per core
IN_DIM = D * (1 + HIST)     # 2304
KC1 = IN_DIM // 128         # 18 contraction chunks for GEMM1
KC2 = HID // 128            # 4 chunks for GEMM2/GEMM3
W3F = D + 1                 # GEMM3 free dim: 256 outputs + row-sum column
GA = 7                      # neurons in the early sqrt group
LN_EPS = 1e-5
FMIN, FMAX = 0.5, 40.0
TICK_INTERVAL = 0.1

f32 = mybir.dt.float32
f16 = mybir.dt.float16

# packed per-neuron row layout for the generic (non-zero bias) path
B1_OFF = 0
B2_OFF = B1_OFF + HID
B3_OFF = B2_OFF + HID
GM_OFF = B3_OFF + W3F
BM_OFF = GM_OFF + D
BVEC_LEN = BM_OFF + D

_CACHE = {}


def _build_program(flags):
    # flags: (b1_zero, b2_zero, b3_zero, gamma_one, beta_zero)
    b1z, b2z, b3z, g1, bz = flags
    nc = bacc.Bacc("TRN2", target_bir_lowering=False, debug=False,
                   num_devices=N_CORES)

    embTd = nc.dram_tensor("embTd", [128, 2, B], f16, kind="ExternalInput").ap()
    wpd = nc.dram_tensor("wpd", [128, 2, D], f16, kind="ExternalInput").ap()
    bpd = nc.dram_tensor("bpd", [128, 2], f32, kind="ExternalInput").ap()
    histd = nc.dram_tensor("histd", [128, HIST * 2], f32, kind="ExternalInput").ap()
    eyed = nc.dram_tensor("eyed", [32, 32], f16, kind="ExternalInput").ap()
    w1d = nc.dram_tensor("w1d", [NPC, 128, KC1, HID], f16, kind="ExternalInput").ap()
    w2d = nc.dram_tensor("w2d", [NPC, 128, KC2, HID], f16, kind="ExternalInput").ap()
    w3d = nc.dram_tensor("w3d", [NPC, 128, KC2, W3F], f16, kind="ExternalInput").ap()
    bvecd = nc.dram_tensor("bvecd", [NPC, BVEC_LEN], f16, kind="ExternalInput").ap()
    sel8d = nc.dram_tensor("sel8d", [NPC, NPC * B], f16, kind="ExternalInput").ap()
    modd = nc.dram_tensor("modd", [B, NPC], f32, kind="ExternalInput").ap()
    out = nc.dram_tensor("out", [B, NPC, D], f32, kind="ExternalOutput").ap()

    GELU = mybir.ActivationFunctionType.Gelu
    SQRT = mybir.ActivationFunctionType.Sqrt
    COPY = mybir.ActivationFunctionType.Copy
    MULT = mybir.AluOpType.mult

    with tile.TileContext(nc) as tc, ExitStack() as ctx:
        cst = ctx.enter_context(tc.tile_pool(name="cst", bufs=1))
        w1p = ctx.enter_context(tc.tile_pool(name="w1p", bufs=8))
        w23p = ctx.enter_context(tc.tile_pool(name="w23p", bufs=8))
        htp = ctx.enter_context(tc.tile_pool(name="htp", bufs=16))
        hp = ctx.enter_context(tc.tile_pool(name="hp", bufs=4))
        ysp = ctx.enter_context(tc.tile_pool(name="ysp", bufs=NPC))
        yop = ctx.enter_context(tc.tile_pool(name="yop", bufs=NPC))
        stp = ctx.enter_context(tc.tile_pool(name="stp", bufs=4))
        scp = ctx.enter_context(tc.tile_pool(name="scp", bufs=2))
        accp = ctx.enter_context(tc.tile_pool(name="accp", bufs=4, space="PSUM"))
        trp = ctx.enter_context(tc.tile_pool(name="trp", bufs=4, space="PSUM"))

        need_sel = not (b1z and b2z and b3z and g1 and bz)

        # embT/wpt lead the fast gpsimd ring: the proj path gates the first
        # GEMM.  Weight pools are deep enough that the DMA stream free-runs
        # ahead while the PE lags on buffered weights.
        embT = cst.tile([128, 2, B], f16, tag="embT")
        nc.gpsimd.dma_start(out=embT[:], in_=embTd)
        wpt = cst.tile([128, 2, D], f16, tag="wpt")
        nc.gpsimd.dma_start(out=wpt[:], in_=wpd)

        eye = cst.tile([32, 32], f16, tag="eye")
        nc.sync.dma_start(out=eye[:], in_=eyed)
        epst = cst.tile([B, 1], f32, tag="epst")
        nc.vector.memset(epst[:], LN_EPS)
        onesb = cst.tile([128, B], f16, tag="onesb")
        nc.vector.memset(onesb[:], 1.0)
        bpt = cst.tile([128, 2], f32, tag="bpt")
        nc.sync.dma_start(out=bpt[:], in_=bpd)
        hist16 = cst.tile([128, HIST * 2], f32, tag="hist16")
        nc.sync.dma_start(out=hist16[:], in_=histd)
        modt = cst.tile([B, NPC], f32, tag="modt")
        nc.sync.dma_start(out=modt[:], in_=modd)
        if need_sel:
            bvec = cst.tile([NPC, BVEC_LEN], f16, tag="bvec")
            nc.sync.dma_start(out=bvec[:], in_=bvecd)
            sel8 = cst.tile([NPC, NPC * B], f16, tag="sel8")
            nc.sync.dma_start(out=sel8[:], in_=sel8d)

            def selcol(n):
                return sel8[:, n * B:(n + 1) * B]

        # ---- weight streaming (gpsimd SWDGE ring). W1 goes in two pieces so
        # GEMM1 can start as soon as the first half lands.
        def dma_w1(n):
            # neuron 0 streams in quarters so the first GEMM starts earlier;
            # later neurons use halves (fewer descriptors in steady state)
            if n == 0:
                ws = []
                for s in range(4):
                    w = w1p.tile([128, 9, HID], f16, tag="w1")
                    nc.gpsimd.dma_start(out=w[:, 0:4, :] if s < 2 else w[:, 0:5, :],
                                        in_=w1d[n][:, [0, 4, 8, 13][s]:[4, 8, 13, 18][s], :])
                    ws.append(w)
                return ("q", ws)
            ws = []
            for s in range(2):
                w = w1p.tile([128, 9, HID], f16, tag="w1")
                nc.gpsimd.dma_start(out=w[:], in_=w1d[n][:, 9 * s:9 * s + 9, :])
                ws.append(w)
            return ("h", ws)

        def dma_w2(n):
            w2t = w23p.tile([128, KC2, HID], f16, tag="w2")
            nc.gpsimd.dma_start(out=w2t[:], in_=w2d[n])
            return w2t

        def dma_w3(n):
            w3t = w23p.tile([128, KC2, W3F], f16, tag="w3")
            nc.gpsimd.dma_start(out=w3t[:], in_=w3d[n])
            return w3t

        # ---- x setup: 18 lhsT chunks [128, 32] f16 ----
        xT = []
        for m in range(2):
            pp = trp.tile([128, 32], f32, tag="tr")
            for k in range(2):
                nc.tensor.matmul(pp[:], wpt[:, k, m * 128:(m + 1) * 128],
                                 embT[:, k, :], start=(k == 0), stop=(k == 1))
            xt = cst.tile([128, 32], f16, tag=f"xt{m}")
            nc.vector.tensor_scalar_add(xt[:], pp[:], bpt[:, m:m + 1])
            xT.append(xt)
        histb = cst.tile([128, HIST * 2, B], f16, tag="histb")
        for c in range(HIST * 2):
            nc.vector.tensor_scalar_mul(histb[:, c, :], onesb[:],
                                        hist16[:, c:c + 1])

        def xchunk(k):
            if k < 2:
                return xT[k][:]
            return histb[:, k - 2, :]

        # ---- pipelined per-neuron schedule ----
        ssqA = cst.tile([B, GA], f32, tag="ssqA")
        ssqB = cst.tile([B, NPC - GA], f32, tag="ssqB")
        ycs = {}
        invA = {}
        invB = {}

        def g1_mm(n, w1t):
            p1 = accp.tile([B, HID], f32, tag="acc")
            if not b1z:
                nc.tensor.matmul(p1[:], selcol(n), bvec[:, B1_OFF:B1_OFF + HID],
                                 start=True, stop=False)
            kind, wts = w1t
            for k in range(KC1):
                if kind == "q":
                    s = 0 if k < 4 else (1 if k < 8 else (2 if k < 13 else 3))
                    base = [0, 4, 8, 13][s]
                    rhs = wts[s][:, k - base, :]
                else:
                    rhs = wts[k // 9][:, k % 9, :]
                nc.tensor.matmul(p1[:], xchunk(k), rhs,
                                 start=(b1z and k == 0), stop=(k == KC1 - 1))
            return p1

        def gelu(p):
            h = hp.tile([B, HID], f16, tag="h")
            nc.scalar.activation(h[:], p[:], GELU)
            return h

        def transpose4(h, dve_drain):
            # dve_drain=True for h2T: its copies gate GEMM3 this step, so they
            # drain on the DVE instead of queueing behind gelus on ACT.  h1T
            # copies are only needed next step and ride ACT.
            hT = []
            for j in range(KC2):
                pt = trp.tile([128, 32], f16, tag="tr")
                nc.tensor.transpose(pt[:], h[:, j * 128:(j + 1) * 128], eye[:])
                st = htp.tile([128, 32], f16, tag="hT")
                if dve_drain:
                    nc.vector.tensor_copy(st[:], pt[:])
                else:
                    nc.scalar.copy(st[:], pt[:])
                hT.append(st)
            return hT

        def g2_mm(n, w2t, h1T):
            p2 = accp.tile([B, HID], f32, tag="acc")
            if not b2z:
                nc.tensor.matmul(p2[:], selcol(n), bvec[:, B2_OFF:B2_OFF + HID],
                                 start=True, stop=False)
            for j in range(KC2):
                nc.tensor.matmul(p2[:], h1T[j][:], w2t[:, j, :],
                                 start=(b2z and j == 0), stop=(j == KC2 - 1))
            return p2

        def g3_mm(n, w3t, h2T):
            p3 = accp.tile([B, W3F], f32, tag="acc")
            if not b3z:
                nc.tensor.matmul(p3[:], selcol(n), bvec[:, B3_OFF:B3_OFF + W3F],
                                 start=True, stop=False)
            for j in range(KC2):
                nc.tensor.matmul(p3[:], h2T[j][:], w3t[:, j, :],
                                 start=(b3z and j == 0), stop=(j == KC2 - 1))
            return p3

        def ln_stats(n, p3):
            nmu = stp.tile([B, 1], f32, tag="st")
            nc.vector.tensor_scalar_mul(nmu[:], p3[:, D:D + 1], -1.0 / D)
            yc = ysp.tile([B, D], f32, tag="ys")
            nc.vector.tensor_scalar_add(yc[:], p3[:, 0:D], nmu[:])
            sq = scp.tile([B, D], f32, tag="sq")
            nc.vector.tensor_tensor(sq[:], yc[:], yc[:], MULT)
            if n < GA:
                nc.vector.tensor_reduce(ssqA[:, n:n + 1], sq[:],
                                        mybir.AxisListType.X, mybir.AluOpType.add)
            else:
                nc.vector.tensor_reduce(ssqB[:, n - GA:n - GA + 1], sq[:],
                                        mybir.AxisListType.X, mybir.AluOpType.add)
            ycs[n] = yc

        def sqrt_group(ssq, width, inv_map, base):
            std = stp.tile([B, width], f32, tag=f"std{base}")
            nc.scalar.activation(std[:], ssq[:], SQRT, bias=epst[:], scale=1.0 / D)
            inv = stp.tile([B, width], f32, tag=f"inv{base}")
            nc.vector.reciprocal(inv[:], std[:])
            if g1 and bz:
                nc.vector.tensor_tensor(inv[:], inv[:],
                                        modt[:, base:base + width], MULT)
            for i in range(width):
                inv_map[base + i] = inv[:, i:i + 1]

        def tail(n, inv_n, dve, dma_eng):
            yc = ycs[n]
            if g1 and bz:
                yo = yop.tile([B, D], f32, tag="yo")
                if dve:
                    nc.vector.tensor_scalar_mul(yo[:], yc[:], inv_n)
                else:
                    nc.scalar.activation(yo[:], yc[:], COPY, scale=inv_n)
            else:
                gb = trp.tile([B, 2 * D], f32, tag="tr")
                nc.tensor.matmul(gb[:, 0:D], selcol(n),
                                 bvec[:, GM_OFF:GM_OFF + D], start=True, stop=True)
                nc.tensor.matmul(gb[:, D:2 * D], selcol(n),
                                 bvec[:, BM_OFF:BM_OFF + D], start=True, stop=True)
                yg = yop.tile([B, D], f32, tag="yo")
                nc.vector.scalar_tensor_tensor(yg[:], yc[:], inv_n, gb[:, 0:D],
                                               MULT, MULT)
                yo = yop.tile([B, D], f32, tag="yo")
                nc.vector.tensor_add(yo[:], yg[:], gb[:, D:2 * D])
            dma_eng.dma_start(out=out[:, n, :], in_=yo[:])

        # pipeline: step n retires neuron n-1 through GEMM2/3 while GEMM1(n)
        # runs; emission order matches the intended per-engine execution
        # order (G2(n-1), G1(n), tr h2(n-1), G3(n-1), tr h1(n)) so the gelus
        # retire in the order the PE consumes them
        h1Ts = {}
        h2Ts = {}
        w2ts = {}
        w3ts = {}
        for n in range(NPC):
            w1t = dma_w1(n)
            w2ts[n] = dma_w2(n)
            w3ts[n] = dma_w3(n)
            if n >= 1:
                p2 = g2_mm(n - 1, w2ts[n - 1], h1Ts[n - 1])
                h2 = gelu(p2)
            p1 = g1_mm(n, w1t)
            h1 = gelu(p1)
            if n >= 1:
                h2Ts[n - 1] = transpose4(h2, True)
                p3 = g3_mm(n - 1, w3ts[n - 1], h2Ts[n - 1])
                ln_stats(n - 1, p3)
            h1Ts[n] = transpose4(h1, False)
            if n - 1 == GA - 1 and GA < NPC - 1:
                sqrt_group(ssqA, GA, invA, 0)
                for i in range(GA):
                    tail(i, invA[i], dve=(i % 2 == 0), dma_eng=nc.sync)
        # epilogue: retire neuron 7.  The early-group sqrt+tails are emitted
        # after gelu(h2(7)) so the tail muls never block it on ACT.
        L = NPC - 1
        p2 = g2_mm(L, w2ts[L], h1Ts[L])
        h2 = gelu(p2)
        if GA == NPC - 1:
            sqrt_group(ssqA, GA, invA, 0)
            for i in range(GA):
                tail(i, invA[i], dve=(i % 2 == 0), dma_eng=nc.sync)
        h2Ts[L] = transpose4(h2, True)
        p3 = g3_mm(L, w3ts[L], h2Ts[L])
        ln_stats(L, p3)
        sqrt_group(ssqB, NPC - GA, invB, GA)
        tail(L, invB[L], dve=True, dma_eng=nc.gpsimd)

    nc.compile()
    return nc


def _get_program(flags):
    if flags not in _CACHE:
        _CACHE[flags] = _build_program(flags)
    return _CACHE[flags]


def _prep(input_embedding, pre_activations, Wp, bp, W1, b1, W2, b2, W3, b3,
          gamma, beta, tick):
    emb = np.asarray(input_embedding, dtype=np.float32)
    hist = np.asarray(pre_activations, dtype=np.float32)
    Wp = np.asarray(Wp, dtype=np.float32)
    bp = np.asarray(bp, dtype=np.float32)
    W1 = np.asarray(W1, dtype=np.float32)
    b1 = np.asarray(b1, dtype=np.float32)
    W2 = np.asarray(W2, dtype=np.float32)
    b2 = np.asarray(b2, dtype=np.float32)
    W3 = np.asarray(W3, dtype=np.float32)
    b3 = np.asarray(b3, dtype=np.float32)
    gamma = np.asarray(gamma, dtype=np.float32)
    beta = np.asarray(beta, dtype=np.float32)

    # oscillator modulation: deterministic in (tick, n); folded into inv_std
    # (gamma==1, beta==0) or into gamma*mod / beta*mod rows otherwise
    i = np.arange(N_NEURONS, dtype=np.float64)
    freq = FMIN * (FMAX / FMIN) ** (i / (N_NEURONS - 1))
    phase = np.mod(i * 2.3571, 2.0 * math.pi)
    t = float(np.asarray(tick)) * TICK_INTERVAL
    mod = (1.0 + 0.5 * np.sin(2.0 * math.pi * freq * t + phase)).astype(np.float32)

    b1z = not np.any(b1)
    b2z = not np.any(b2)
    b3z = not np.any(b3)
    g1 = bool(np.all(gamma == 1.0))
    bz = not np.any(beta)

    # fp16 weight layouts: (n, partition, k_chunk, free) with contiguous
    # per-partition runs; W3 gains a row-sum column so the GEMM also
    # produces sum_d(y) for the LayerNorm mean
    W1r = np.ascontiguousarray(
        W1.reshape(N_NEURONS, KC1, 128, HID).transpose(0, 2, 1, 3)).astype(np.float16)
    W2r = np.ascontiguousarray(
        W2.reshape(N_NEURONS, KC2, 128, HID).transpose(0, 2, 1, 3)).astype(np.float16)
    W3a = np.concatenate([W3, W3.sum(axis=2, keepdims=True)], axis=2)
    W3r = np.ascontiguousarray(
        W3a.reshape(N_NEURONS, KC2, 128, W3F).transpose(0, 2, 1, 3)).astype(np.float16)

    embT = np.ascontiguousarray(emb.T.reshape(2, 128, B).transpose(1, 0, 2)).astype(np.float16)
    wpt = np.ascontiguousarray(Wp.reshape(2, 128, D).transpose(1, 0, 2)).astype(np.float16)
    bpd = np.ascontiguousarray(bp.reshape(2, 128).T)
    hist16 = np.ascontiguousarray(hist.reshape(-1).reshape(16, 128).T)  # [128, 16]
    eyed = np.eye(32, dtype=np.float16)

    gm = (gamma * mod[:, None]).astype(np.float32)
    bm = (beta * mod[:, None]).astype(np.float32)
    b3a = np.concatenate([b3, b3.sum(axis=1, keepdims=True)], axis=1)
    sel8 = np.zeros((NPC, NPC * B), dtype=np.float16)
    for n in range(NPC):
        sel8[n, n * B:(n + 1) * B] = 1.0

    in_maps = []
    for c in range(N_CORES):
        s = slice(c * NPC, (c + 1) * NPC)
        bvec = np.concatenate([b1[s], b2[s], b3a[s], gm[s], bm[s]],
                              axis=1).astype(np.float16)
        modrow = np.broadcast_to(mod[c * NPC:(c + 1) * NPC][None, :],
                                 (B, NPC)).astype(np.float32)
        in_maps.append({
            "modd": np.ascontiguousarray(modrow),
            "embTd": embT,
            "wpd": wpt,
            "bpd": bpd,
            "histd": hist16,
            "eyed": eyed,
            "w1d": W1r[s],
            "w2d": W2r[s],
            "w3d": W3r[s],
            "bvecd": np.ascontiguousarray(bvec),
            "sel8d": sel8,
        })
    flags = (b1z, b2z, b3z, g1, bz)
    return in_maps, flags


def run(inputs, trace=False):
    in_maps, flags = _prep(**inputs)
    nc = _get_program(flags)
    br = run_bass_kernel_spmd(nc, in_maps, core_ids=list(range(N_CORES)),
                              trace=trace)
    out = np.concatenate([r["out"] for r in br.results], axis=1)
    return np.ascontiguousarray(out, dtype=np.float32), br


def kernel(**inputs) -> np.ndarray:
    out, _ = run(inputs, trace=False)
    return out


# revision 48
# speedup vs baseline: 1.0057x; 1.0057x over previous
"""NeuronPool (moe_routing) Trainium2 kernel.

Expert-parallel over 8 NeuronCores: core c computes neurons [8c, 8c+8) for the
full batch, host concatenates along the neuron axis.

The kernel is HBM-bound: ~25 MB of fp16 weights stream per core at ~420 GB/s
(weights cast on host; matmuls run at the full 1 cycle/row PE rate, ~5e-4
relative rounding).  The per-neuron MLPs are software-pipelined one neuron
deep so the PE never waits on gelu: step n runs GEMM1(n) while GEMM2/GEMM3 of
neuron n-1 fill the gelu/transpose latency.

  x = [proj | hist]: proj = Wp.T @ embT on the PE; hist chunks broadcast
      across batch on the DVE.  W1/W2 weights stream on the fast gpsimd SWDGE
      ring (16 DMA engines); small constants ride the sync ring.
  step n:  G2(n-1) ; G1(n) ; tr4 h2(n-1) ; G3(n-1) ; tr4 h1(n), with the
           two gelus retiring on the scalar engine in consumption order
  GEMM3 output is [32,257]: col 256 = row-sums of W3 (host-augmented), so the
  LayerNorm mean is a free by-product of the GEMM.
  LN scale: sqrt over neurons {0..6} runs while neuron 7 computes (their
  outputs drain early); only neuron 7 remains in the tail.  The oscillator
  mod is folded into inv_std via one DVE multiply with a per-core constant.
Zero bias vectors / unit gamma / zero beta are detected on host at call time
and their device ops are skipped (generic selector-matmul paths are emitted
when the values are non-trivial).
"""
import math
import numpy as np
from contextlib import ExitStack

import concourse.bass as bass
import concourse.tile as tile
from concourse import bacc, mybir
from concourse.bass_utils import run_bass_kernel_spmd

N_CORES = 8
B = 32          # batch
D = 256         # model dim
HIST = 8
HID = 512
N_NEURONS = 64
NPC = N_NEURONS // N_CORES  # 8 neurons per core
IN_DIM = D * (1 + HIST)     # 2304
KC1 = IN_DIM // 128         # 18 contraction chunks for GEMM1
KC2 = HID // 128            # 4 chunks for GEMM2/GEMM3
W3F = D + 1                 # GEMM3 free dim: 256 outputs + row-sum column
GA = 7                      # neurons in the early sqrt group
LN_EPS = 1e-5
FMIN, FMAX = 0.5, 40.0
TICK_INTERVAL = 0.1

f32 = mybir.dt.float32
f16 = mybir.dt.float16

# packed per-neuron row layout for the generic (non-zero bias) path
B1_OFF = 0
B2_OFF = B1_OFF + HID
B3_OFF = B2_OFF + HID
GM_OFF = B3_OFF + W3F
BM_OFF = GM_OFF + D
BVEC_LEN = BM_OFF + D

_CACHE = {}


def _build_program(flags):
    # flags: (b1_zero, b2_zero, b3_zero, gamma_one, beta_zero)
    b1z, b2z, b3z, g1, bz = flags
    nc = bacc.Bacc("TRN2", target_bir_lowering=False, debug=False,
                   num_devices=N_CORES)

    embTd = nc.dram_tensor("embTd", [128, 2, B], f16, kind="ExternalInput").ap()
    wpd = nc.dram_tensor("wpd", [128, 2, D], f16, kind="ExternalInput").ap()
    bpd = nc.dram_tensor("bpd", [128, 2], f32, kind="ExternalInput").ap()
    histd = nc.dram_tensor("histd", [128, HIST * 2], f32, kind="ExternalInput").ap()
    eyed = nc.dram_tensor("eyed", [32, 32], f16, kind="ExternalInput").ap()
    w1d = nc.dram_tensor("w1d", [NPC, 128, KC1, HID], f16, kind="ExternalInput").ap()
    w2d = nc.dram_tensor("w2d", [NPC, 128, KC2, HID], f16, kind="ExternalInput").ap()
    w3d = nc.dram_tensor("w3d", [NPC, 128, KC2, W3F], f16, kind="ExternalInput").ap()
    bvecd = nc.dram_tensor("bvecd", [NPC, BVEC_LEN], f16, kind="ExternalInput").ap()
    sel8d = nc.dram_tensor("sel8d", [NPC, NPC * B], f16, kind="ExternalInput").ap()
    modd = nc.dram_tensor("modd", [B, NPC], f32, kind="ExternalInput").ap()
    out = nc.dram_tensor("out", [B, NPC, D], f32, kind="ExternalOutput").ap()

    GELU = mybir.ActivationFunctionType.Gelu
    SQRT = mybir.ActivationFunctionType.Sqrt
    COPY = mybir.ActivationFunctionType.Copy
    MULT = mybir.AluOpType.mult

    with tile.TileContext(nc) as tc, ExitStack() as ctx:
        cst = ctx.enter_context(tc.tile_pool(name="cst", bufs=1))
        w1p = ctx.enter_context(tc.tile_pool(name="w1p", bufs=8))
        w23p = ctx.enter_context(tc.tile_pool(name="w23p", bufs=8))
        htp = ctx.enter_context(tc.tile_pool(name="htp", bufs=16))
        hp = ctx.enter_context(tc.tile_pool(name="hp", bufs=4))
        ysp = ctx.enter_context(tc.tile_pool(name="ysp", bufs=NPC))
        yop = ctx.enter_context(tc.tile_pool(name="yop", bufs=NPC))
        stp = ctx.enter_context(tc.tile_pool(name="stp", bufs=4))
        scp = ctx.enter_context(tc.tile_pool(name="scp", bufs=2))
        accp = ctx.enter_context(tc.tile_pool(name="accp", bufs=4, space="PSUM"))
        trp = ctx.enter_context(tc.tile_pool(name="trp", bufs=4, space="PSUM"))

        need_sel = not (b1z and b2z and b3z and g1 and bz)

        # embT/wpt lead the fast gpsimd ring: the proj path gates the first
        # GEMM.  Weight pools are deep enough that the DMA stream free-runs
        # ahead while the PE lags on buffered weights.
        embT = cst.tile([128, 2, B], f16, tag="embT")
        nc.gpsimd.dma_start(out=embT[:], in_=embTd)
        wpt = cst.tile([128, 2, D], f16, tag="wpt")
        nc.gpsimd.dma_start(out=wpt[:], in_=wpd)

        eye = cst.tile([32, 32], f16, tag="eye")
        nc.sync.dma_start(out=eye[:], in_=eyed)
        epst = cst.tile([B, 1], f32, tag="epst")
        nc.vector.memset(epst[:], LN_EPS)
        onesb = cst.tile([128, B], f16, tag="onesb")
        nc.vector.memset(onesb[:], 1.0)
        bpt = cst.tile([128, 2], f32, tag="bpt")
        nc.sync.dma_start(out=bpt[:], in_=bpd)
        hist16 = cst.tile([128, HIST * 2], f32, tag="hist16")
        nc.sync.dma_start(out=hist16[:], in_=histd)
        modt = cst.tile([B, NPC], f32, tag="modt")
        nc.sync.dma_start(out=modt[:], in_=modd)
        if need_sel:
            bvec = cst.tile([NPC, BVEC_LEN], f16, tag="bvec")
            nc.sync.dma_start(out=bvec[:], in_=bvecd)
            sel8 = cst.tile([NPC, NPC * B], f16, tag="sel8")
            nc.sync.dma_start(out=sel8[:], in_=sel8d)

            def selcol(n):
                return sel8[:, n * B:(n + 1) * B]

        # ---- weight streaming (gpsimd SWDGE ring). W1 goes in two pieces so
        # GEMM1 can start as soon as the first half lands.
        def dma_w1(n):
            # neuron 0 streams in quarters so the first GEMM starts earlier;
            # later neurons use halves (fewer descriptors in steady state)
            if n == 0:
                ws = []
                for s in range(4):
                    w = w1p.tile([128, 9, HID], f16, tag="w1")
                    nc.gpsimd.dma_start(out=w[:, 0:4, :] if s < 2 else w[:, 0:5, :],
                                        in_=w1d[n][:, [0, 4, 8, 13][s]:[4, 8, 13, 18][s], :])
                    ws.append(w)
                return ("q", ws)
            ws = []
            for s in range(2):
                w = w1p.tile([128, 9, HID], f16, tag="w1")
                nc.gpsimd.dma_start(out=w[:], in_=w1d[n][:, 9 * s:9 * s + 9, :])
                ws.append(w)
            return ("h", ws)

        def dma_w2(n):
            w2t = w23p.tile([128, KC2, HID], f16, tag="w2")
            nc.gpsimd.dma_start(out=w2t[:], in_=w2d[n])
            return w2t

        def dma_w3(n):
            w3t = w23p.tile([128, KC2, W3F], f16, tag="w3")
            nc.gpsimd.dma_start(out=w3t[:], in_=w3d[n])
            return w3t

        # ---- x setup: 18 lhsT chunks [128, 32] f16 ----
        xT = []
        for m in range(2):
            pp = trp.tile([128, 32], f32, tag="tr")
            for k in range(2):
                nc.tensor.matmul(pp[:], wpt[:, k, m * 128:(m + 1) * 128],
                                 embT[:, k, :], start=(k == 0), stop=(k == 1))
            xt = cst.tile([128, 32], f16, tag=f"xt{m}")
            nc.vector.tensor_scalar_add(xt[:], pp[:], bpt[:, m:m + 1])
            xT.append(xt)
        histb = cst.tile([128, HIST * 2, B], f16, tag="histb")
        for c in range(HIST * 2):
            nc.vector.tensor_scalar_mul(histb[:, c, :], onesb[:],
                                        hist16[:, c:c + 1])

        def xchunk(k):
            if k < 2:
                return xT[k][:]
            return histb[:, k - 2, :]

        # ---- pipelined per-neuron schedule ----
        ssqA = cst.tile([B, GA], f32, tag="ssqA")
        ssqB = cst.tile([B, NPC - GA], f32, tag="ssqB")
        ycs = {}
        invA = {}
        invB = {}

        def g1_mm(n, w1t):
            # one PSUM bank, TWO accumulation groups (chunks 0-8 close a
            # group, 9-17 continue with start=False): the scheduler treats
            # groups atomically, so group A runs as soon as the first W1
            # piece lands instead of waiting for the whole W1
            kind, wts = w1t
            p1 = accp.tile([B, HID], f32, tag="acc")
            if not b1z:
                nc.tensor.matmul(p1[:], selcol(n), bvec[:, B1_OFF:B1_OFF + HID],
                                 start=True, stop=False)
            for k in range(KC1):
                if kind == "q":
                    s = 0 if k < 4 else (1 if k < 8 else (2 if k < 13 else 3))
                    base = [0, 4, 8, 13][s]
                    rhs = wts[s][:, k - base, :]
                else:
                    rhs = wts[k // 9][:, k % 9, :]
                nc.tensor.matmul(p1[:], xchunk(k), rhs,
                                 start=(b1z and k == 0),
                                 stop=(k == 8 or k == KC1 - 1),
                                 skip_group_check=(k > 8))
            return p1

        def gelu(p):
            h = hp.tile([B, HID], f16, tag="h")
            nc.scalar.activation(h[:], p[:], GELU)
            return h

        def transpose4(h, dve_drain):
            # dve_drain=True for h2T: its copies gate GEMM3 this step, so they
            # drain on the DVE instead of queueing behind gelus on ACT.  h1T
            # copies are only needed next step and ride ACT.
            hT = []
            for j in range(KC2):
                pt = trp.tile([128, 32], f16, tag="tr")
                nc.tensor.transpose(pt[:], h[:, j * 128:(j + 1) * 128], eye[:])
                st = htp.tile([128, 32], f16, tag="hT")
                if dve_drain:
                    nc.vector.tensor_copy(st[:], pt[:])
                else:
                    nc.scalar.copy(st[:], pt[:])
                hT.append(st)
            return hT

        def g2_mm(n, w2t, h1T):
            p2 = accp.tile([B, HID], f32, tag="acc")
            if not b2z:
                nc.tensor.matmul(p2[:], selcol(n), bvec[:, B2_OFF:B2_OFF + HID],
                                 start=True, stop=False)
            for j in range(KC2):
                nc.tensor.matmul(p2[:], h1T[j][:], w2t[:, j, :],
                                 start=(b2z and j == 0), stop=(j == KC2 - 1))
            return p2

        def g3_mm(n, w3t, h2T):
            p3 = accp.tile([B, W3F], f32, tag="acc")
            if not b3z:
                nc.tensor.matmul(p3[:], selcol(n), bvec[:, B3_OFF:B3_OFF + W3F],
                                 start=True, stop=False)
            for j in range(KC2):
                nc.tensor.matmul(p3[:], h2T[j][:], w3t[:, j, :],
                                 start=(b3z and j == 0), stop=(j == KC2 - 1))
            return p3

        def ln_stats(n, p3):
            nmu = stp.tile([B, 1], f32, tag="st")
            nc.vector.tensor_scalar_mul(nmu[:], p3[:, D:D + 1], -1.0 / D)
            yc = ysp.tile([B, D], f32, tag="ys")
            nc.vector.tensor_scalar_add(yc[:], p3[:, 0:D], nmu[:])
            sq = scp.tile([B, D], f32, tag="sq")
            nc.vector.tensor_tensor(sq[:], yc[:], yc[:], MULT)
            if n < GA:
                nc.vector.tensor_reduce(ssqA[:, n:n + 1], sq[:],
                                        mybir.AxisListType.X, mybir.AluOpType.add)
            else:
                nc.vector.tensor_reduce(ssqB[:, n - GA:n - GA + 1], sq[:],
                                        mybir.AxisListType.X, mybir.AluOpType.add)
            ycs[n] = yc

        def sqrt_group(ssq, width, inv_map, base):
            std = stp.tile([B, width], f32, tag=f"std{base}")
            nc.scalar.activation(std[:], ssq[:], SQRT, bias=epst[:], scale=1.0 / D)
            inv = stp.tile([B, width], f32, tag=f"inv{base}")
            nc.vector.reciprocal(inv[:], std[:])
            if g1 and bz:
                nc.vector.tensor_tensor(inv[:], inv[:],
                                        modt[:, base:base + width], MULT)
            for i in range(width):
                inv_map[base + i] = inv[:, i:i + 1]

        def tail(n, inv_n, dve, dma_eng):
            yc = ycs[n]
            if g1 and bz:
                yo = yop.tile([B, D], f32, tag="yo")
                if dve:
                    nc.vector.tensor_scalar_mul(yo[:], yc[:], inv_n)
                else:
                    nc.scalar.activation(yo[:], yc[:], COPY, scale=inv_n)
            else:
                gb = trp.tile([B, 2 * D], f32, tag="tr")
                nc.tensor.matmul(gb[:, 0:D], selcol(n),
                                 bvec[:, GM_OFF:GM_OFF + D], start=True, stop=True)
                nc.tensor.matmul(gb[:, D:2 * D], selcol(n),
                                 bvec[:, BM_OFF:BM_OFF + D], start=True, stop=True)
                yg = yop.tile([B, D], f32, tag="yo")
                nc.vector.scalar_tensor_tensor(yg[:], yc[:], inv_n, gb[:, 0:D],
                                               MULT, MULT)
                yo = yop.tile([B, D], f32, tag="yo")
                nc.vector.tensor_add(yo[:], yg[:], gb[:, D:2 * D])
            dma_eng.dma_start(out=out[:, n, :], in_=yo[:])

        # pipeline: step n retires neuron n-1 through GEMM2/3 while GEMM1(n)
        # runs; emission order matches the intended per-engine execution
        # order (G2(n-1), G1(n), tr h2(n-1), G3(n-1), tr h1(n)) so the gelus
        # retire in the order the PE consumes them
        h1Ts = {}
        h2Ts = {}
        w2ts = {}
        w3ts = {}
        for n in range(NPC):
            w1t = dma_w1(n)
            w2ts[n] = dma_w2(n)
            w3ts[n] = dma_w3(n)
            if n >= 1:
                p2 = g2_mm(n - 1, w2ts[n - 1], h1Ts[n - 1])
                h2 = gelu(p2)
            p1 = g1_mm(n, w1t)
            h1 = gelu(p1)
            if n >= 1:
                h2Ts[n - 1] = transpose4(h2, True)
                p3 = g3_mm(n - 1, w3ts[n - 1], h2Ts[n - 1])
                ln_stats(n - 1, p3)
            h1Ts[n] = transpose4(h1, False)
            if n - 1 == GA - 1 and GA < NPC - 1:
                sqrt_group(ssqA, GA, invA, 0)
                for i in range(GA):
                    tail(i, invA[i], dve=(i % 2 == 0), dma_eng=nc.sync)
        # epilogue: retire neuron 7.  The early-group sqrt+tails are emitted
        # after gelu(h2(7)) so the tail muls never block it on ACT.
        L = NPC - 1
        p2 = g2_mm(L, w2ts[L], h1Ts[L])
        h2 = gelu(p2)
        if GA == NPC - 1:
            sqrt_group(ssqA, GA, invA, 0)
            for i in range(GA):
                tail(i, invA[i], dve=(i % 2 == 0), dma_eng=nc.sync)
        h2Ts[L] = transpose4(h2, True)
        p3 = g3_mm(L, w3ts[L], h2Ts[L])
        ln_stats(L, p3)
        sqrt_group(ssqB, NPC - GA, invB, GA)
        tail(L, invB[L], dve=True, dma_eng=nc.gpsimd)

    nc.compile()
    return nc


def _get_program(flags):
    if flags not in _CACHE:
        _CACHE[flags] = _build_program(flags)
    return _CACHE[flags]


def _prep(input_embedding, pre_activations, Wp, bp, W1, b1, W2, b2, W3, b3,
          gamma, beta, tick):
    emb = np.asarray(input_embedding, dtype=np.float32)
    hist = np.asarray(pre_activations, dtype=np.float32)
    Wp = np.asarray(Wp, dtype=np.float32)
    bp = np.asarray(bp, dtype=np.float32)
    W1 = np.asarray(W1, dtype=np.float32)
    b1 = np.asarray(b1, dtype=np.float32)
    W2 = np.asarray(W2, dtype=np.float32)
    b2 = np.asarray(b2, dtype=np.float32)
    W3 = np.asarray(W3, dtype=np.float32)
    b3 = np.asarray(b3, dtype=np.float32)
    gamma = np.asarray(gamma, dtype=np.float32)
    beta = np.asarray(beta, dtype=np.float32)

    # oscillator modulation: deterministic in (tick, n); folded into inv_std
    # (gamma==1, beta==0) or into gamma*mod / beta*mod rows otherwise
    i = np.arange(N_NEURONS, dtype=np.float64)
    freq = FMIN * (FMAX / FMIN) ** (i / (N_NEURONS - 1))
    phase = np.mod(i * 2.3571, 2.0 * math.pi)
    t = float(np.asarray(tick)) * TICK_INTERVAL
    mod = (1.0 + 0.5 * np.sin(2.0 * math.pi * freq * t + phase)).astype(np.float32)

    b1z = not np.any(b1)
    b2z = not np.any(b2)
    b3z = not np.any(b3)
    g1 = bool(np.all(gamma == 1.0))
    bz = not np.any(beta)

    # fp16 weight layouts: (n, partition, k_chunk, free) with contiguous
    # per-partition runs; W3 gains a row-sum column so the GEMM also
    # produces sum_d(y) for the LayerNorm mean
    W1r = np.ascontiguousarray(
        W1.reshape(N_NEURONS, KC1, 128, HID).transpose(0, 2, 1, 3)).astype(np.float16)
    W2r = np.ascontiguousarray(
        W2.reshape(N_NEURONS, KC2, 128, HID).transpose(0, 2, 1, 3)).astype(np.float16)
    W3a = np.concatenate([W3, W3.sum(axis=2, keepdims=True)], axis=2)
    W3r = np.ascontiguousarray(
        W3a.reshape(N_NEURONS, KC2, 128, W3F).transpose(0, 2, 1, 3)).astype(np.float16)

    embT = np.ascontiguousarray(emb.T.reshape(2, 128, B).transpose(1, 0, 2)).astype(np.float16)
    wpt = np.ascontiguousarray(Wp.reshape(2, 128, D).transpose(1, 0, 2)).astype(np.float16)
    bpd = np.ascontiguousarray(bp.reshape(2, 128).T)
    hist16 = np.ascontiguousarray(hist.reshape(-1).reshape(16, 128).T)  # [128, 16]
    eyed = np.eye(32, dtype=np.float16)

    gm = (gamma * mod[:, None]).astype(np.float32)
    bm = (beta * mod[:, None]).astype(np.float32)
    b3a = np.concatenate([b3, b3.sum(axis=1, keepdims=True)], axis=1)
    sel8 = np.zeros((NPC, NPC * B), dtype=np.float16)
    for n in range(NPC):
        sel8[n, n * B:(n + 1) * B] = 1.0

    in_maps = []
    for c in range(N_CORES):
        s = slice(c * NPC, (c + 1) * NPC)
        bvec = np.concatenate([b1[s], b2[s], b3a[s], gm[s], bm[s]],
                              axis=1).astype(np.float16)
        modrow = np.broadcast_to(mod[c * NPC:(c + 1) * NPC][None, :],
                                 (B, NPC)).astype(np.float32)
        in_maps.append({
            "modd": np.ascontiguousarray(modrow),
            "embTd": embT,
            "wpd": wpt,
            "bpd": bpd,
            "histd": hist16,
            "eyed": eyed,
            "w1d": W1r[s],
            "w2d": W2r[s],
            "w3d": W3r[s],
            "bvecd": np.ascontiguousarray(bvec),
            "sel8d": sel8,
        })
    flags = (b1z, b2z, b3z, g1, bz)
    return in_maps, flags


def run(inputs, trace=False):
    in_maps, flags = _prep(**inputs)
    nc = _get_program(flags)
    br = run_bass_kernel_spmd(nc, in_maps, core_ids=list(range(N_CORES)),
                              trace=trace)
    out = np.concatenate([r["out"] for r in br.results], axis=1)
    return np.ascontiguousarray(out, dtype=np.float32), br


def kernel(**inputs) -> np.ndarray:
    out, _ = run(inputs, trace=False)
    return out


# revision 49
# speedup vs baseline: 1.0444x; 1.0385x over previous
"""NeuronPool (moe_routing) Trainium2 kernel.

Expert-parallel over 8 NeuronCores: core c computes neurons [8c, 8c+8) for the
full batch, host concatenates along the neuron axis.

The kernel is HBM-bound: ~25 MB of fp16 weights stream per core at ~420 GB/s
(weights cast on host; matmuls run at the full 1 cycle/row PE rate, ~5e-4
relative rounding).  The per-neuron MLPs are software-pipelined one neuron
deep so the PE never waits on gelu: step n runs GEMM1(n) while GEMM2/GEMM3 of
neuron n-1 fill the gelu/transpose latency.

  x = [proj | hist]: proj = Wp.T @ embT on the PE; hist chunks broadcast
      across batch on the DVE.  W1/W2 weights stream on the fast gpsimd SWDGE
      ring (16 DMA engines); small constants ride the sync ring.
  step n:  G2(n-1) ; G1(n) ; tr4 h2(n-1) ; G3(n-1) ; tr4 h1(n), with the
           two gelus retiring on the scalar engine in consumption order
  GEMM3 output is [32,257]: col 256 = row-sums of W3 (host-augmented), so the
  LayerNorm mean is a free by-product of the GEMM.
  LN scale: sqrt over neurons {0..6} runs while neuron 7 computes (their
  outputs drain early); only neuron 7 remains in the tail.  The oscillator
  mod is folded into inv_std via one DVE multiply with a per-core constant.
Zero bias vectors / unit gamma / zero beta are detected on host at call time
and their device ops are skipped (generic selector-matmul paths are emitted
when the values are non-trivial).
"""
import math
import numpy as np
from contextlib import ExitStack

import concourse.bass as bass
import concourse.tile as tile
from concourse import bacc, mybir
from concourse.bass_utils import run_bass_kernel_spmd

N_CORES = 8
B = 32          # batch
D = 256         # model dim
HIST = 8
HID = 512
N_NEURONS = 64
NPC = N_NEURONS // N_CORES  # 8 neurons per core
IN_DIM = D * (1 + HIST)     # 2304
KC1 = IN_DIM // 128         # 18 contraction chunks for GEMM1
KC2 = HID // 128            # 4 chunks for GEMM2/GEMM3
W3F = D + 1                 # GEMM3 free dim: 256 outputs + row-sum column
GA = 7                      # neurons in the early sqrt group
LN_EPS = 1e-5
FMIN, FMAX = 0.5, 40.0
TICK_INTERVAL = 0.1

f32 = mybir.dt.float32
f16 = mybir.dt.float16

# packed per-neuron row layout for the generic (non-zero bias) path
B1_OFF = 0
B2_OFF = B1_OFF + HID
B3_OFF = B2_OFF + HID
GM_OFF = B3_OFF + W3F
BM_OFF = GM_OFF + D
BVEC_LEN = BM_OFF + D

_CACHE = {}


def _build_program(flags):
    # flags: (b1_zero, b2_zero, b3_zero, gamma_one, beta_zero)
    b1z, b2z, b3z, g1, bz = flags
    nc = bacc.Bacc("TRN2", target_bir_lowering=False, debug=False,
                   num_devices=N_CORES)

    embTd = nc.dram_tensor("embTd", [128, 2, B], f16, kind="ExternalInput").ap()
    wpd = nc.dram_tensor("wpd", [128, 2, D], f16, kind="ExternalInput").ap()
    bpd = nc.dram_tensor("bpd", [128, 2], f32, kind="ExternalInput").ap()
    histd = nc.dram_tensor("histd", [128, HIST * 2], f32, kind="ExternalInput").ap()
    eyed = nc.dram_tensor("eyed", [32, 32], f16, kind="ExternalInput").ap()
    w1d = nc.dram_tensor("w1d", [NPC, 128, KC1, HID], f16, kind="ExternalInput").ap()
    w2d = nc.dram_tensor("w2d", [NPC, 128, KC2, HID], f16, kind="ExternalInput").ap()
    w3d = nc.dram_tensor("w3d", [NPC, 128, KC2, W3F], f16, kind="ExternalInput").ap()
    bvecd = nc.dram_tensor("bvecd", [NPC, BVEC_LEN], f16, kind="ExternalInput").ap()
    sel8d = nc.dram_tensor("sel8d", [NPC, NPC * B], f16, kind="ExternalInput").ap()
    modd = nc.dram_tensor("modd", [B, NPC], f32, kind="ExternalInput").ap()
    out = nc.dram_tensor("out", [B, NPC, D], f32, kind="ExternalOutput").ap()

    GELU = mybir.ActivationFunctionType.Gelu
    SQRT = mybir.ActivationFunctionType.Sqrt
    COPY = mybir.ActivationFunctionType.Copy
    MULT = mybir.AluOpType.mult

    with tile.TileContext(nc) as tc, ExitStack() as ctx:
        cst = ctx.enter_context(tc.tile_pool(name="cst", bufs=1))
        w1p = ctx.enter_context(tc.tile_pool(name="w1p", bufs=8))
        w23p = ctx.enter_context(tc.tile_pool(name="w23p", bufs=8))
        htp = ctx.enter_context(tc.tile_pool(name="htp", bufs=16))
        hp = ctx.enter_context(tc.tile_pool(name="hp", bufs=4))
        ysp = ctx.enter_context(tc.tile_pool(name="ysp", bufs=NPC))
        yop = ctx.enter_context(tc.tile_pool(name="yop", bufs=NPC))
        stp = ctx.enter_context(tc.tile_pool(name="stp", bufs=4))
        scp = ctx.enter_context(tc.tile_pool(name="scp", bufs=2))
        accp = ctx.enter_context(tc.tile_pool(name="accp", bufs=4, space="PSUM"))
        trp = ctx.enter_context(tc.tile_pool(name="trp", bufs=4, space="PSUM"))

        need_sel = not (b1z and b2z and b3z and g1 and bz)

        # embT/wpt lead the fast gpsimd ring: the proj path gates the first
        # GEMM.  Weight pools are deep enough that the DMA stream free-runs
        # ahead while the PE lags on buffered weights.
        embT = cst.tile([128, 2, B], f16, tag="embT")
        nc.gpsimd.dma_start(out=embT[:], in_=embTd)
        wpt = cst.tile([128, 2, D], f16, tag="wpt")
        nc.gpsimd.dma_start(out=wpt[:], in_=wpd)

        eye = cst.tile([32, 32], f16, tag="eye")
        nc.sync.dma_start(out=eye[:], in_=eyed)
        epst = cst.tile([B, 1], f32, tag="epst")
        nc.vector.memset(epst[:], LN_EPS)
        onesb = cst.tile([128, B], f16, tag="onesb")
        nc.vector.memset(onesb[:], 1.0)
        bpt = cst.tile([128, 2], f32, tag="bpt")
        nc.sync.dma_start(out=bpt[:], in_=bpd)
        hist16 = cst.tile([128, HIST * 2], f32, tag="hist16")
        nc.sync.dma_start(out=hist16[:], in_=histd)
        modt = cst.tile([B, NPC], f32, tag="modt")
        nc.sync.dma_start(out=modt[:], in_=modd)
        if need_sel:
            bvec = cst.tile([NPC, BVEC_LEN], f16, tag="bvec")
            nc.sync.dma_start(out=bvec[:], in_=bvecd)
            sel8 = cst.tile([NPC, NPC * B], f16, tag="sel8")
            nc.sync.dma_start(out=sel8[:], in_=sel8d)

            def selcol(n):
                return sel8[:, n * B:(n + 1) * B]

        # ---- weight streaming (gpsimd SWDGE ring). W1 goes in two pieces so
        # GEMM1 can start as soon as the first half lands.
        def dma_w1(n):
            # neuron 0 streams in quarters so the first GEMM starts earlier;
            # later neurons use halves (fewer descriptors in steady state)
            if n == 0:
                ws = []
                for s in range(4):
                    w = w1p.tile([128, 9, HID], f16, tag="w1")
                    nc.gpsimd.dma_start(out=w[:, 0:4, :] if s < 2 else w[:, 0:5, :],
                                        in_=w1d[n][:, [0, 4, 8, 13][s]:[4, 8, 13, 18][s], :])
                    ws.append(w)
                return ("q", ws)
            ws = []
            for s in range(2):
                w = w1p.tile([128, 9, HID], f16, tag="w1")
                nc.gpsimd.dma_start(out=w[:], in_=w1d[n][:, 9 * s:9 * s + 9, :])
                ws.append(w)
            return ("h", ws)

        def dma_w2(n):
            w2t = w23p.tile([128, KC2, HID], f16, tag="w2")
            nc.gpsimd.dma_start(out=w2t[:], in_=w2d[n])
            return w2t

        def dma_w3(n):
            w3t = w23p.tile([128, KC2, W3F], f16, tag="w3")
            nc.gpsimd.dma_start(out=w3t[:], in_=w3d[n])
            return w3t

        # ---- x setup: 18 lhsT chunks [128, 32] f16 ----
        xT = []
        for m in range(2):
            pp = trp.tile([128, 32], f32, tag="tr")
            for k in range(2):
                nc.tensor.matmul(pp[:], wpt[:, k, m * 128:(m + 1) * 128],
                                 embT[:, k, :], start=(k == 0), stop=(k == 1))
            xt = cst.tile([128, 32], f16, tag=f"xt{m}")
            nc.vector.tensor_scalar_add(xt[:], pp[:], bpt[:, m:m + 1])
            xT.append(xt)
        histb = cst.tile([128, HIST * 2, B], f16, tag="histb")
        for c in range(HIST * 2):
            nc.vector.tensor_scalar_mul(histb[:, c, :], onesb[:],
                                        hist16[:, c:c + 1])

        def xchunk(k):
            if k < 2:
                return xT[k][:]
            return histb[:, k - 2, :]

        # ---- pipelined per-neuron schedule ----
        ssqA = cst.tile([B, GA], f32, tag="ssqA")
        ssqB = cst.tile([B, NPC - GA], f32, tag="ssqB")
        ycs = {}
        invA = {}
        invB = {}

        def g1_mm(n, w1t):
            p1 = accp.tile([B, HID], f32, tag="acc")
            if not b1z:
                nc.tensor.matmul(p1[:], selcol(n), bvec[:, B1_OFF:B1_OFF + HID],
                                 start=True, stop=False)
            kind, wts = w1t
            for k in range(KC1):
                if kind == "q":
                    s = 0 if k < 4 else (1 if k < 8 else (2 if k < 13 else 3))
                    base = [0, 4, 8, 13][s]
                    rhs = wts[s][:, k - base, :]
                else:
                    rhs = wts[k // 9][:, k % 9, :]
                nc.tensor.matmul(p1[:], xchunk(k), rhs,
                                 start=(b1z and k == 0), stop=(k == KC1 - 1))
            return p1

        def gelu(p):
            h = hp.tile([B, HID], f16, tag="h")
            nc.scalar.activation(h[:], p[:], GELU)
            return h

        def transpose4(h, dve_drain):
            # dve_drain=True for h2T: its copies gate GEMM3 this step, so they
            # drain on the DVE instead of queueing behind gelus on ACT.  h1T
            # copies are only needed next step and ride ACT.
            hT = []
            for j in range(KC2):
                pt = trp.tile([128, 32], f16, tag="tr")
                nc.tensor.transpose(pt[:], h[:, j * 128:(j + 1) * 128], eye[:])
                st = htp.tile([128, 32], f16, tag="hT")
                if dve_drain:
                    nc.vector.tensor_copy(st[:], pt[:])
                else:
                    nc.scalar.copy(st[:], pt[:])
                hT.append(st)
            return hT

        def g2_mm(n, w2t, h1T):
            p2 = accp.tile([B, HID], f32, tag="acc")
            if not b2z:
                nc.tensor.matmul(p2[:], selcol(n), bvec[:, B2_OFF:B2_OFF + HID],
                                 start=True, stop=False)
            for j in range(KC2):
                nc.tensor.matmul(p2[:], h1T[j][:], w2t[:, j, :],
                                 start=(b2z and j == 0), stop=(j == KC2 - 1))
            return p2

        def g3_mm(n, w3t, h2T):
            p3 = accp.tile([B, W3F], f32, tag="acc")
            if not b3z:
                nc.tensor.matmul(p3[:], selcol(n), bvec[:, B3_OFF:B3_OFF + W3F],
                                 start=True, stop=False)
            for j in range(KC2):
                nc.tensor.matmul(p3[:], h2T[j][:], w3t[:, j, :],
                                 start=(b3z and j == 0), stop=(j == KC2 - 1))
            return p3

        def ln_stats(n, p3):
            nmu = stp.tile([B, 1], f32, tag="st")
            nc.vector.tensor_scalar_mul(nmu[:], p3[:, D:D + 1], -1.0 / D)
            yc = ysp.tile([B, D], f32, tag="ys")
            nc.vector.tensor_scalar_add(yc[:], p3[:, 0:D], nmu[:])
            sq = scp.tile([B, D], f32, tag="sq")
            nc.vector.tensor_tensor(sq[:], yc[:], yc[:], MULT)
            if n < GA:
                nc.vector.tensor_reduce(ssqA[:, n:n + 1], sq[:],
                                        mybir.AxisListType.X, mybir.AluOpType.add)
            else:
                nc.vector.tensor_reduce(ssqB[:, n - GA:n - GA + 1], sq[:],
                                        mybir.AxisListType.X, mybir.AluOpType.add)
            ycs[n] = yc

        def sqrt_group(ssq, width, inv_map, base):
            std = stp.tile([B, width], f32, tag=f"std{base}")
            nc.scalar.activation(std[:], ssq[:], SQRT, bias=epst[:], scale=1.0 / D)
            inv = stp.tile([B, width], f32, tag=f"inv{base}")
            nc.vector.reciprocal(inv[:], std[:])
            if g1 and bz:
                nc.vector.tensor_tensor(inv[:], inv[:],
                                        modt[:, base:base + width], MULT)
            for i in range(width):
                inv_map[base + i] = inv[:, i:i + 1]

        def tail(n, inv_n, dve, dma_eng):
            yc = ycs[n]
            if g1 and bz:
                yo = yop.tile([B, D], f32, tag="yo")
                if dve:
                    nc.vector.tensor_scalar_mul(yo[:], yc[:], inv_n)
                else:
                    nc.scalar.activation(yo[:], yc[:], COPY, scale=inv_n)
            else:
                gb = trp.tile([B, 2 * D], f32, tag="tr")
                nc.tensor.matmul(gb[:, 0:D], selcol(n),
                                 bvec[:, GM_OFF:GM_OFF + D], start=True, stop=True)
                nc.tensor.matmul(gb[:, D:2 * D], selcol(n),
                                 bvec[:, BM_OFF:BM_OFF + D], start=True, stop=True)
                yg = yop.tile([B, D], f32, tag="yo")
                nc.vector.scalar_tensor_tensor(yg[:], yc[:], inv_n, gb[:, 0:D],
                                               MULT, MULT)
                yo = yop.tile([B, D], f32, tag="yo")
                nc.vector.tensor_add(yo[:], yg[:], gb[:, D:2 * D])
            dma_eng.dma_start(out=out[:, n, :], in_=yo[:])

        # pipeline: step n retires neuron n-1 through GEMM2/3 while GEMM1(n)
        # runs; emission order matches the intended per-engine execution
        # order (G2(n-1), G1(n), tr h2(n-1), G3(n-1), tr h1(n)) so the gelus
        # retire in the order the PE consumes them
        h1Ts = {}
        h2Ts = {}
        w2ts = {}
        w3ts = {}
        for n in range(NPC):
            w1t = dma_w1(n)
            w2ts[n] = dma_w2(n)
            w3ts[n] = dma_w3(n)
            if n >= 1:
                p2 = g2_mm(n - 1, w2ts[n - 1], h1Ts[n - 1])
                h2 = gelu(p2)
            p1 = g1_mm(n, w1t)
            h1 = gelu(p1)
            if n >= 1:
                h2Ts[n - 1] = transpose4(h2, True)
                p3 = g3_mm(n - 1, w3ts[n - 1], h2Ts[n - 1])
                ln_stats(n - 1, p3)
            h1Ts[n] = transpose4(h1, False)
            if n - 1 == GA - 1 and GA < NPC - 1:
                sqrt_group(ssqA, GA, invA, 0)
                for i in range(GA):
                    tail(i, invA[i], dve=(i % 2 == 0), dma_eng=nc.sync)
        # epilogue: retire neuron 7.  The early-group sqrt+tails are emitted
        # after gelu(h2(7)) so the tail muls never block it on ACT.
        L = NPC - 1
        p2 = g2_mm(L, w2ts[L], h1Ts[L])
        h2 = gelu(p2)
        if GA == NPC - 1:
            sqrt_group(ssqA, GA, invA, 0)
            for i in range(GA):
                tail(i, invA[i], dve=(i % 2 == 0), dma_eng=nc.sync)
        h2Ts[L] = transpose4(h2, True)
        p3 = g3_mm(L, w3ts[L], h2Ts[L])
        ln_stats(L, p3)
        sqrt_group(ssqB, NPC - GA, invB, GA)
        tail(L, invB[L], dve=True, dma_eng=nc.gpsimd)

    nc.compile()
    return nc


def _get_program(flags):
    if flags not in _CACHE:
        _CACHE[flags] = _build_program(flags)
    return _CACHE[flags]


def _prep(input_embedding, pre_activations, Wp, bp, W1, b1, W2, b2, W3, b3,
          gamma, beta, tick):
    emb = np.asarray(input_embedding, dtype=np.float32)
    hist = np.asarray(pre_activations, dtype=np.float32)
    Wp = np.asarray(Wp, dtype=np.float32)
    bp = np.asarray(bp, dtype=np.float32)
    W1 = np.asarray(W1, dtype=np.float32)
    b1 = np.asarray(b1, dtype=np.float32)
    W2 = np.asarray(W2, dtype=np.float32)
    b2 = np.asarray(b2, dtype=np.float32)
    W3 = np.asarray(W3, dtype=np.float32)
    b3 = np.asarray(b3, dtype=np.float32)
    gamma = np.asarray(gamma, dtype=np.float32)
    beta = np.asarray(beta, dtype=np.float32)

    # oscillator modulation: deterministic in (tick, n); folded into inv_std
    # (gamma==1, beta==0) or into gamma*mod / beta*mod rows otherwise
    i = np.arange(N_NEURONS, dtype=np.float64)
    freq = FMIN * (FMAX / FMIN) ** (i / (N_NEURONS - 1))
    phase = np.mod(i * 2.3571, 2.0 * math.pi)
    t = float(np.asarray(tick)) * TICK_INTERVAL
    mod = (1.0 + 0.5 * np.sin(2.0 * math.pi * freq * t + phase)).astype(np.float32)

    b1z = not np.any(b1)
    b2z = not np.any(b2)
    b3z = not np.any(b3)
    g1 = bool(np.all(gamma == 1.0))
    bz = not np.any(beta)

    # fp16 weight layouts: (n, partition, k_chunk, free) with contiguous
    # per-partition runs; W3 gains a row-sum column so the GEMM also
    # produces sum_d(y) for the LayerNorm mean
    W1r = np.ascontiguousarray(
        W1.reshape(N_NEURONS, KC1, 128, HID).transpose(0, 2, 1, 3)).astype(np.float16)
    W2r = np.ascontiguousarray(
        W2.reshape(N_NEURONS, KC2, 128, HID).transpose(0, 2, 1, 3)).astype(np.float16)
    W3a = np.concatenate([W3, W3.sum(axis=2, keepdims=True)], axis=2)
    W3r = np.ascontiguousarray(
        W3a.reshape(N_NEURONS, KC2, 128, W3F).transpose(0, 2, 1, 3)).astype(np.float16)

    embT = np.ascontiguousarray(emb.T.reshape(2, 128, B).transpose(1, 0, 2)).astype(np.float16)
    wpt = np.ascontiguousarray(Wp.reshape(2, 128, D).transpose(1, 0, 2)).astype(np.float16)
    bpd = np.ascontiguousarray(bp.reshape(2, 128).T)
    hist16 = np.ascontiguousarray(hist.reshape(-1).reshape(16, 128).T)  # [128, 16]
    eyed = np.eye(32, dtype=np.float16)

    gm = (gamma * mod[:, None]).astype(np.float32)
    bm = (beta * mod[:, None]).astype(np.float32)
    b3a = np.concatenate([b3, b3.sum(axis=1, keepdims=True)], axis=1)
    sel8 = np.zeros((NPC, NPC * B), dtype=np.float16)
    for n in range(NPC):
        sel8[n, n * B:(n + 1) * B] = 1.0

    in_maps = []
    for c in range(N_CORES):
        s = slice(c * NPC, (c + 1) * NPC)
        bvec = np.concatenate([b1[s], b2[s], b3a[s], gm[s], bm[s]],
                              axis=1).astype(np.float16)
        modrow = np.broadcast_to(mod[c * NPC:(c + 1) * NPC][None, :],
                                 (B, NPC)).astype(np.float32)
        in_maps.append({
            "modd": np.ascontiguousarray(modrow),
            "embTd": embT,
            "wpd": wpt,
            "bpd": bpd,
            "histd": hist16,
            "eyed": eyed,
            "w1d": W1r[s],
            "w2d": W2r[s],
            "w3d": W3r[s],
            "bvecd": np.ascontiguousarray(bvec),
            "sel8d": sel8,
        })
    flags = (b1z, b2z, b3z, g1, bz)
    return in_maps, flags


def run(inputs, trace=False):
    in_maps, flags = _prep(**inputs)
    nc = _get_program(flags)
    br = run_bass_kernel_spmd(nc, in_maps, core_ids=list(range(N_CORES)),
                              trace=trace)
    out = np.concatenate([r["out"] for r in br.results], axis=1)
    return np.ascontiguousarray(out, dtype=np.float32), br


def kernel(**inputs) -> np.ndarray:
    out, _ = run(inputs, trace=False)
    return out
